# revision 11
# baseline (speedup 1.0000x reference)
"""DocRED relation-extraction head on 8 Trainium2 NeuronCores.

Data-parallel over the batch axis: core b owns batch b's hidden_states slab
and its entity/pair indices; classifier weights are replicated.

The model has NO nonlinearity between the two linear layers:

    logits = (concat(subj, obj) @ dense_w + dense_b) @ out_w + out_b
           = subj @ (W1 @ out_w) + obj @ (W2 @ out_w) + (dense_b @ out_w + out_b)

so the classifier folds into A = dense_w @ out_w [2H, C] (host sgemm, one
weight-only transform) and the device never streams the 8MB dense_w at all.
Device work per core collapses to:

    gather  128 mention rows of hidden_states via indirect DMA   (256 KB)
    repT    mention-sum fused with transpose: 8 matmuls against a
            block-ones matrix -> entT chunks [128h, 8x32e]
    eL      8 accumulating matmuls vs packed fused-A chunks [A1c|A2c]
            ([128,32]^T @ [128,196] -> [32,196] PSUM)             (410 KB)
    logits  stacked one-hot pair gather: rows 0-31 head one-hot,
            32-63 tail, 64 all-ones (adds cvec); 8 matmuls
            [65,128]^T @ [65,98] -> [128,98] per pair tile        (133 KB in,
                                                                   401 KB out)

All matmul inputs travel fp16 (magnitudes well inside range; single-pass PE);
PSUM accumulation is fp32. End-to-end vs the fp32 reference: ~4e-4 relative.

Latency structure (the kernel is bound by serial latencies, not bandwidth):
the critical chain is pos-DMA completion (~2.5us) -> indirect-gather
descriptor gen + data (~2.5us) -> stages A/B/D (~2.5us) -> output DMA
completion (~2.5us), bracketed by ~8us of fixed framework preamble/teardown.
Stage A and D matmuls write disjoint column regions of a shared PSUM bank
(start=True on the first matmul clears the whole bank; later matmuls write
fresh onto cleared has_written bits), so each bank drains with ONE wide
PSUM->SBUF copy instead of eight. A f32 dummy-matmul burn spans the gather
window to release the HAM clock gate (idle PE held at half clock) right as
stage A starts, keeping all real matmuls at 2.4 GHz.
"""

import numpy as np
from contextlib import ExitStack

import concourse.bass as bass
import concourse.bacc as bacc
import concourse.tile as tile
import concourse.mybir as mybir
from concourse.bass_utils import run_bass_kernel_spmd

B, L, H, E, M, P, C = 8, 2048, 1024, 32, 4, 1024, 97
N_CORES = 8
HC = H // 128   # h-dim chunks (contraction of the fused projection)
PT = P // 128   # pair tiles
CP = C + 1      # class dim padded to 98 (even moving dim; pad col zero)
KD = 2 * E + 1  # stage-D contraction: 32 head + 32 tail + 1 const row

f32 = mybir.dt.float32
f16 = mybir.dt.float16
i32 = mybir.dt.int32

AB_CVEC = HC * 2 * CP           # cvec column block offset in ablob
AB_ONES = AB_CVEC + CP          # ones-block column offset
ABLOBW = AB_ONES + E            # 8 x [A1c | A2c] + cvec + ones-block

_CACHE = {}


def _build():
    nc = bacc.Bacc("TRN2", target_bir_lowering=False, debug=False)

    hs = nc.dram_tensor("hs", [L, H], f16, kind="ExternalInput").ap()
    pos = nc.dram_tensor("pos", [E * M, 1], i32, kind="ExternalInput").ap()
    ablob = nc.dram_tensor("ablob", [128, ABLOBW], f16, kind="ExternalInput").ap()
    oh = nc.dram_tensor("oh", [KD, P], f16, kind="ExternalInput").ap()
    # output laid out [128, PT*CP]: pair-tile t in columns t*CP..(t+1)*CP,
    # pad column included; fp16 halves the tail DMA (host strips + upcasts)
    out = nc.dram_tensor("out", [128, PT * CP], f16, kind="ExternalOutput").ap()

    with tile.TileContext(nc) as tc, ExitStack() as ctx:
        sb = ctx.enter_context(tc.tile_pool(name="sb", bufs=1))
        opool = ctx.enter_context(tc.tile_pool(name="o", bufs=2))
        pspool = ctx.enter_context(tc.tile_pool(name="ps", bufs=8, space="PSUM"))

        # ---- input DMAs, all on the sync ring: pos first (the gather blocks
        # on its completion sem), then the two stage-B/D blobs. The scalar
        # ring stays empty so ACT's table load never delays pos.
        sb_pos = sb.tile([E * M, 1], i32)
        nc.sync.dma_start(sb_pos[:], pos[:])
        sb_ablob = sb.tile([128, ABLOBW], f16)
        nc.sync.dma_start(sb_ablob[:], ablob[:])
        sb_oh = sb.tile([KD, P], f16)
        nc.sync.dma_start(sb_oh[:], oh[:])

        # ---- gather the 128 mention rows of hidden_states
        sb_g = sb.tile([E * M, H], f16)
        nc.gpsimd.indirect_dma_start(
            out=sb_g[:],
            out_offset=None,
            in_=hs[:],
            in_offset=bass.IndirectOffsetOnAxis(ap=sb_pos[:, :1], axis=0),
        )

        # ---- PE warm-up: f32 dummies (2 ISA passes, ~107ns each) spanning
        # the pos+gather window so the HAM clock gate releases right as
        # stage A starts and the PE never re-idles into a throttle window.
        wdum = sb.tile([128, E], f32)
        nc.vector.memset(wdum[:], 0.0)
        # dummy ACT op: forces the framework's 1.3us ACT_TABLE_LOAD to run
        # now (behind the input DMAs) instead of lazily in front of the
        # first real ACT copy on the critical path
        sb_dummy = sb.tile([1, 2], f16)
        nc.scalar.copy(sb_dummy[:], wdum[:1, :2])
        ps_warm = pspool.tile([E, E], f32, tag="ps")
        for i in range(54):
            nc.tensor.matmul(
                out=ps_warm[:],
                lhsT=wdum[:],
                rhs=wdum[:],
                start=True,
                stop=True,
            )

        # ---- stage A: entity_repT[h, e] = sum_m gathered[4e+m, h]
        # (mention-sum and transpose fused into 8 matmuls vs the ones-block).
        # All 8 write disjoint 32-col regions of ONE PSUM bank; one wide copy.
        ps_rep = pspool.tile([128, HC * E], f32, tag="ps", name="ps_rep")
        for hc in range(HC):
            nc.tensor.matmul(
                out=ps_rep[:, hc * E:(hc + 1) * E],
                lhsT=sb_g[:, hc * 128:(hc + 1) * 128],
                rhs=sb_ablob[:, AB_ONES:AB_ONES + E],
                start=(hc == 0),
                stop=(hc == HC - 1),
                skip_group_check=True,
            )
        # drain in two halves so stage B's first matmuls start one cast early
        sb_repT = sb.tile([128, HC * E], f16)
        nc.vector.tensor_copy(out=sb_repT[:, :HC * E // 2],
                              in_=ps_rep[:, :HC * E // 2])
        nc.vector.tensor_copy(out=sb_repT[:, HC * E // 2:],
                              in_=ps_rep[:, HC * E // 2:])

        # ---- stage B: eL = entT^T @ [A1 | A2] accumulated over h chunks.
        # ps_eL cols 0:98 = eL1 (head-role), 98:196 = eL2 (tail-role).
        ps_eL = pspool.tile([E, 2 * CP], f32, tag="ps", name="ps_eL")
        for hc in range(HC):
            nc.tensor.matmul(
                out=ps_eL[:],
                lhsT=sb_repT[:, hc * E:(hc + 1) * E],
                rhs=sb_ablob[:, hc * 2 * CP:(hc + 1) * 2 * CP],
                start=(hc == 0),
                stop=(hc == HC - 1),
            )

        # ---- eL stack [65, 98]: rows 0-31 eL1, 32-63 eL2, row 64 = cvec
        sb_eL = sb.tile([KD, CP], f16)
        nc.vector.tensor_copy(
            out=sb_eL[2 * E:2 * E + 1, :],
            in_=sb_ablob[:1, AB_CVEC:AB_CVEC + CP])
        nc.vector.tensor_copy(out=sb_eL[:E, :], in_=ps_eL[:, :CP])
        nc.scalar.copy(sb_eL[E:2 * E, :], ps_eL[:, CP:2 * CP])

        # ---- stage D: stacked one-hot pair gather, 128 pairs per matmul.
        # Tiles 0-3 / 4-7 fill two PSUM banks as 98-col regions; each bank
        # drains with one wide copy + one half-output DMA. The second half's
        # copy and DMA both sit on the ACT queue (no cross-engine hop).
        sb_out = opool.tile([128, PT * CP], f16, bufs=1)
        ps_half = [pspool.tile([128, 4 * CP], f32, tag="ps", name=f"psd{h}")
                   for h in range(2)]
        for pt in range(PT):
            h, q = divmod(pt, 4)
            nc.tensor.matmul(
                out=ps_half[h][:, q * CP:(q + 1) * CP],
                lhsT=sb_oh[:, pt * 128:(pt + 1) * 128],
                rhs=sb_eL[:],
                start=(q == 0),
                stop=(q == 3),
                skip_group_check=True,
            )
        nc.vector.tensor_copy(out=sb_out[:, :4 * CP], in_=ps_half[0][:])
        nc.sync.dma_start(out[:, :4 * CP], sb_out[:, :4 * CP])
        nc.scalar.copy(sb_out[:, 4 * CP:], ps_half[1][:])
        nc.scalar.dma_start(out[:, 4 * CP:], sb_out[:, 4 * CP:])

    nc.compile()
    return nc


def get_compiled():
    if "nc" not in _CACHE:
        _CACHE["nc"] = _build()
    return _CACHE["nc"]


def make_in_maps(hidden_states, dense_w, dense_b, out_w, out_b,
                 entity_position_ids, head_tail_idxs):
    # inputs may arrive as jax arrays; normalize to host numpy first
    hidden_states = np.asarray(hidden_states)
    dense_w = np.asarray(dense_w, np.float32)
    dense_b = np.asarray(dense_b, np.float32)
    out_w = np.asarray(out_w, np.float32)
    out_b = np.asarray(out_b, np.float32)
    entity_position_ids = np.asarray(entity_position_ids)
    head_tail_idxs = np.asarray(head_tail_idxs)

    # fold the classifier: A = dense_w @ out_w, cvec = dense_b @ out_w + out_b
    A = dense_w @ out_w                       # [2H, C]
    cvec = dense_b @ out_w + out_b            # [C]
    Ap = np.zeros((2 * H, CP), np.float32)
    Ap[:, :C] = A
    ablob = np.zeros((128, ABLOBW), np.float16)
    for hc in range(HC):
        ablob[:, hc * 2 * CP:hc * 2 * CP + CP] = \
            Ap[hc * 128:(hc + 1) * 128]                      # A1 chunk
        ablob[:, hc * 2 * CP + CP:(hc + 1) * 2 * CP] = \
            Ap[H + hc * 128:H + (hc + 1) * 128]              # A2 chunk
    ablob[0, AB_CVEC:AB_CVEC + C] = cvec.astype(np.float16)
    ablob[:, AB_ONES:AB_ONES + E] = \
        np.repeat(np.eye(E, dtype=np.float16), M, axis=0)    # ones-block

    in_maps = []
    for b in range(B):
        ht = head_tail_idxs[b]                # [P, 2]
        oh = np.zeros((KD, P), np.float16)
        oh[ht[:, 0], np.arange(P)] = 1.0      # head one-hot rows 0-31
        oh[E + ht[:, 1], np.arange(P)] = 1.0  # tail one-hot rows 32-63
        oh[2 * E, :] = 1.0                    # const row
        in_maps.append({
            "hs": np.ascontiguousarray(hidden_states[b], dtype=np.float16),
            "pos": np.ascontiguousarray(
                entity_position_ids[b].reshape(E * M, 1).astype(np.int32)),
            "ablob": ablob,
            "oh": oh,
        })
    return in_maps


def kernel(hidden_states, dense_w, dense_b, out_w, out_b,
           entity_position_ids, head_tail_idxs, _trace=False, _trace_kwargs=None):
    nc = get_compiled()
    in_maps = make_in_maps(hidden_states, dense_w, dense_b, out_w, out_b,
                           entity_position_ids, head_tail_idxs)
    res = run_bass_kernel_spmd(
        nc, in_maps, core_ids=list(range(N_CORES)),
        trace=_trace, **(_trace_kwargs or {}),
    )
    outp = np.concatenate(
        [res.results[i]["out"].astype(np.float32).reshape(128, PT, CP)[:, :, :C]
         .transpose(1, 0, 2).reshape(P, C) for i in range(N_CORES)], axis=0)
    if _trace:
        return outp, res
    return outp


# revision 20
# speedup vs baseline: 1.0973x; 1.0973x over previous
"""DocRED relation-extraction head on 8 Trainium2 NeuronCores.

Data-parallel over the batch axis: core b owns batch b's hidden_states slab
and its entity/pair indices; classifier weights are replicated.

The model has NO nonlinearity between the two linear layers:

    logits = (concat(subj, obj) @ dense_w + dense_b) @ out_w + out_b
           = subj @ (W1 @ out_w) + obj @ (W2 @ out_w) + (dense_b @ out_w + out_b)

so the classifier folds into A = dense_w @ out_w [2H, C] (host sgemm, one
weight-only transform) and the device never streams the 8MB dense_w at all.
Device work per core collapses to:

    gather  128 mention rows of hidden_states via indirect DMA   (256 KB)
    repT    mention-sum fused with transpose: 8 matmuls against a
            block-ones matrix -> entT chunks [128h, 8x32e]
    eL      8 accumulating matmuls vs packed fused-A chunks [A1c|A2c]
            ([128,32]^T @ [128,196] -> [32,196] PSUM)             (410 KB)
    logits  stacked one-hot pair gather: rows 0-31 head one-hot,
            32-63 tail, 64 all-ones (adds cvec); 8 matmuls
            [65,128]^T @ [65,98] -> [128,98] per pair tile        (133 KB in,
                                                                   401 KB out)

All matmul inputs travel fp16 (magnitudes well inside range; single-pass PE);
PSUM accumulation is fp32. End-to-end vs the fp32 reference: ~4e-4 relative.

Latency structure (the kernel is bound by serial latencies, not bandwidth):
the critical chain is pos-DMA completion (~2.5us) -> indirect-gather
descriptor gen + data (~2.5us) -> stages A/B/D (~2.5us) -> output DMA
completion (~2.5us), bracketed by ~8us of fixed framework preamble/teardown.
Stage A and D matmuls write disjoint column regions of a shared PSUM bank
(start=True on the first matmul clears the whole bank; later matmuls write
fresh onto cleared has_written bits), so each bank drains with ONE wide
PSUM->SBUF copy instead of eight. A f32 dummy-matmul burn spans the gather
window to release the HAM clock gate (idle PE held at half clock) right as
stage A starts, keeping all real matmuls at 2.4 GHz.
"""

import numpy as np
from contextlib import ExitStack

import concourse.bass as bass
import concourse.bacc as bacc
import concourse.tile as tile
import concourse.mybir as mybir
from concourse.bass_utils import run_bass_kernel_spmd

B, L, H, E, M, P, C = 8, 2048, 1024, 32, 4, 1024, 97
N_CORES = 8
HC = H // 128   # h-dim chunks (contraction of the fused projection)
PT = P // 128   # pair tiles
CP = C + 1      # class dim padded to 98 (even moving dim; pad col zero)
KD = 2 * E + 1  # stage-D contraction: 32 head + 32 tail + 1 const row

f32 = mybir.dt.float32
f16 = mybir.dt.float16
i32 = mybir.dt.int32

AB_CVEC = HC * 2 * CP           # cvec column block offset in ablob
AB_ONES = AB_CVEC + CP          # ones-block for gather half 0 (rows 0-63)
AB_ONES2 = AB_ONES + E          # ones-block rows 64-127, stored on rows 0-63
ABLOBW = AB_ONES2 + E           # (matmul operands must share base partition)

_CACHE = {}


def _build():
    nc = bacc.Bacc("TRN2", target_bir_lowering=False, debug=False)

    hs = nc.dram_tensor("hs", [L, H], f16, kind="ExternalInput").ap()
    pos = nc.dram_tensor("pos", [E * M, 1], i32, kind="ExternalInput").ap()
    ablob = nc.dram_tensor("ablob", [128, ABLOBW], f16, kind="ExternalInput").ap()
    oh = nc.dram_tensor("oh", [KD, P], f16, kind="ExternalInput").ap()
    # output laid out [128, PT*CP]: pair-tile t in columns t*CP..(t+1)*CP,
    # pad column included; fp16 halves the tail DMA (host strips + upcasts)
    out = nc.dram_tensor("out", [128, PT * CP], f16, kind="ExternalOutput").ap()

    with tile.TileContext(nc) as tc, ExitStack() as ctx:
        sb = ctx.enter_context(tc.tile_pool(name="sb", bufs=1))
        opool = ctx.enter_context(tc.tile_pool(name="o", bufs=2))
        pspool = ctx.enter_context(tc.tile_pool(name="ps", bufs=8, space="PSUM"))

        # ---- input DMAs, all on the sync ring: pos first (the gather blocks
        # on its completion sem), then the two stage-B/D blobs. The scalar
        # ring stays empty so ACT's table load never delays pos.
        sb_pos = sb.tile([E * M, 1], i32)
        nc.sync.dma_start(sb_pos[:], pos[:])
        sb_ablob = sb.tile([128, ABLOBW], f16)
        nc.sync.dma_start(sb_ablob[:], ablob[:])
        sb_oh = sb.tile([KD, P], f16)
        nc.sync.dma_start(sb_oh[:], oh[:])

        # ---- gather the 128 mention rows of hidden_states
        # (two back-to-back indirect DMAs wedge the exec unit
        # [NRT_EXEC_UNIT_UNRECOVERABLE]; keep the gather monolithic)
        sb_g = sb.tile([E * M, H], f16)
        nc.gpsimd.indirect_dma_start(
            out=sb_g[:],
            out_offset=None,
            in_=hs[:],
            in_offset=bass.IndirectOffsetOnAxis(ap=sb_pos[:, :1], axis=0),
        )

        # ---- PE warm-up: f32 dummies (2 ISA passes, ~107ns each) spanning
        # the pos+gather window so the HAM clock gate releases right as
        # stage A starts and the PE never re-idles into a throttle window.
        wdum = sb.tile([128, E], f32)
        nc.vector.memset(wdum[:], 0.0)
        # dummy ACT op: forces the framework's 1.3us ACT_TABLE_LOAD to run
        # now (behind the input DMAs) instead of lazily in front of the
        # first real ACT copy on the critical path
        sb_dummy = sb.tile([1, 2], f16)
        nc.scalar.copy(sb_dummy[:], wdum[:1, :2])
        ps_warm = pspool.tile([E, E], f32, tag="ps")
        for i in range(58):
            nc.tensor.matmul(
                out=ps_warm[:],
                lhsT=wdum[:],
                rhs=wdum[:],
                start=True,
                stop=True,
            )

        # ---- stage A: entity_repT[h, e] = sum_m gathered[4e+m, h]
        # (mention-sum and transpose fused into matmuls vs the ones-block).
        # All writes land in disjoint 32-col regions of ONE PSUM bank (the
        # first matmul's start=True clears the whole bank; the rest write
        # fresh onto cleared has_written bits / accumulate the second half);
        # the bank drains with one wide copy per 128-col half.
        ps_rep = pspool.tile([128, HC * E], f32, tag="ps", name="ps_rep")
        for hc in range(HC):
            nc.tensor.matmul(
                out=ps_rep[:, hc * E:(hc + 1) * E],
                lhsT=sb_g[:, hc * 128:(hc + 1) * 128],
                rhs=sb_ablob[:, AB_ONES:AB_ONES + E],
                start=(hc == 0),
                stop=(hc == HC - 1),
                skip_group_check=True,
            )
        # drain in two halves so stage B's first matmuls start one cast early
        sb_repT = sb.tile([128, HC * E], f16)
        nc.vector.tensor_copy(out=sb_repT[:, :HC * E // 2],
                              in_=ps_rep[:, :HC * E // 2])
        nc.vector.tensor_copy(out=sb_repT[:, HC * E // 2:],
                              in_=ps_rep[:, HC * E // 2:])

        # ---- stage B: eL = entT^T @ [A1 | A2] accumulated over h chunks.
        # ps_eL cols 0:98 = eL1 (head-role), 98:196 = eL2 (tail-role).
        ps_eL = pspool.tile([E, 2 * CP], f32, tag="ps", name="ps_eL")
        for hc in range(HC):
            nc.tensor.matmul(
                out=ps_eL[:],
                lhsT=sb_repT[:, hc * E:(hc + 1) * E],
                rhs=sb_ablob[:, hc * 2 * CP:(hc + 1) * 2 * CP],
                start=(hc == 0),
                stop=(hc == HC - 1),
            )

        # ---- eL stack [65, 98]: rows 0-31 eL1, 32-63 eL2, row 64 = cvec
        sb_eL = sb.tile([KD, CP], f16)
        nc.vector.tensor_copy(
            out=sb_eL[2 * E:2 * E + 1, :],
            in_=sb_ablob[:1, AB_CVEC:AB_CVEC + CP])
        nc.vector.tensor_copy(out=sb_eL[:E, :], in_=ps_eL[:, :CP])
        nc.scalar.copy(sb_eL[E:2 * E, :], ps_eL[:, CP:2 * CP])

        # ---- stage D: stacked one-hot pair gather, 128 pairs per matmul.
        # Tiles 0-3 / 4-7 fill two PSUM banks as 98-col regions; each bank
        # drains with one wide copy + one half-output DMA. The second half's
        # copy and DMA both sit on the ACT queue (no cross-engine hop).
        sb_out = opool.tile([128, PT * CP], f16, bufs=1)
        ps_half = [pspool.tile([128, 4 * CP], f32, tag="ps", name=f"psd{h}")
                   for h in range(2)]
        for pt in range(PT):
            h, q = divmod(pt, 4)
            nc.tensor.matmul(
                out=ps_half[h][:, q * CP:(q + 1) * CP],
                lhsT=sb_oh[:, pt * 128:(pt + 1) * 128],
                rhs=sb_eL[:],
                start=(q == 0),
                stop=(q == 3),
                skip_group_check=True,
            )
        nc.vector.tensor_copy(out=sb_out[:, :4 * CP], in_=ps_half[0][:])
        nc.sync.dma_start(out[:, :4 * CP], sb_out[:, :4 * CP])
        nc.scalar.copy(sb_out[:, 4 * CP:], ps_half[1][:])
        nc.scalar.dma_start(out[:, 4 * CP:], sb_out[:, 4 * CP:])

    nc.compile()
    return nc


def get_compiled():
    if "nc" not in _CACHE:
        _CACHE["nc"] = _build()
    return _CACHE["nc"]


def make_in_maps(hidden_states, dense_w, dense_b, out_w, out_b,
                 entity_position_ids, head_tail_idxs):
    # inputs may arrive as jax arrays; normalize to host numpy first
    hidden_states = np.asarray(hidden_states)
    dense_w = np.asarray(dense_w, np.float32)
    dense_b = np.asarray(dense_b, np.float32)
    out_w = np.asarray(out_w, np.float32)
    out_b = np.asarray(out_b, np.float32)
    entity_position_ids = np.asarray(entity_position_ids)
    head_tail_idxs = np.asarray(head_tail_idxs)

    # fold the classifier: A = dense_w @ out_w, cvec = dense_b @ out_w + out_b
    A = dense_w @ out_w                       # [2H, C]
    cvec = dense_b @ out_w + out_b            # [C]
    Ap = np.zeros((2 * H, CP), np.float32)
    Ap[:, :C] = A
    ablob = np.zeros((128, ABLOBW), np.float16)
    for hc in range(HC):
        ablob[:, hc * 2 * CP:hc * 2 * CP + CP] = \
            Ap[hc * 128:(hc + 1) * 128]                      # A1 chunk
        ablob[:, hc * 2 * CP + CP:(hc + 1) * 2 * CP] = \
            Ap[H + hc * 128:H + (hc + 1) * 128]              # A2 chunk
    ablob[0, AB_CVEC:AB_CVEC + C] = cvec.astype(np.float16)
    ablob[:, AB_ONES:AB_ONES + E] = \
        np.repeat(np.eye(E, dtype=np.float16), M, axis=0)    # ones-block

    in_maps = []
    for b in range(B):
        ht = head_tail_idxs[b]                # [P, 2]
        oh = np.zeros((KD, P), np.float16)
        oh[ht[:, 0], np.arange(P)] = 1.0      # head one-hot rows 0-31
        oh[E + ht[:, 1], np.arange(P)] = 1.0  # tail one-hot rows 32-63
        oh[2 * E, :] = 1.0                    # const row
        in_maps.append({
            "hs": np.ascontiguousarray(hidden_states[b], dtype=np.float16),
            "pos": np.ascontiguousarray(
                entity_position_ids[b].reshape(E * M, 1).astype(np.int32)),
            "ablob": ablob,
            "oh": oh,
        })
    return in_maps


def kernel(hidden_states, dense_w, dense_b, out_w, out_b,
           entity_position_ids, head_tail_idxs, _trace=False, _trace_kwargs=None):
    nc = get_compiled()
    in_maps = make_in_maps(hidden_states, dense_w, dense_b, out_w, out_b,
                           entity_position_ids, head_tail_idxs)
    res = run_bass_kernel_spmd(
        nc, in_maps, core_ids=list(range(N_CORES)),
        trace=_trace, **(_trace_kwargs or {}),
    )
    outp = np.concatenate(
        [res.results[i]["out"].astype(np.float32).reshape(128, PT, CP)[:, :, :C]
         .transpose(1, 0, 2).reshape(P, C) for i in range(N_CORES)], axis=0)
    if _trace:
        return outp, res
    return outp
